# revision 2
# baseline (speedup 1.0000x reference)
"""ComplexAttention (B=2, T=2048, D=1024, H=16, Dh=64) on 8 TRN2 NeuronCores.

Sharding: core c -> batch b = c // 4, heads [4*(c%4), 4*(c%4)+4).
Each core computes its 4 heads' QKV projections (column-sharded), causal
complex attention, and a partial output projection (row-sharded). The host
sums the 4 partials per batch and adds the output bias.

bf16 version (tolerance is 2e-2; bf16 matmuls stream 1 cyc/row vs 4 for
fp32 on the PE). Key layout tricks vs the fp32 baseline:
  - x is transposed AND cast to bf16 on the host: the device only ever needs
    x^T (Q/K rhs and V lhsT), so no PE transposes at all.
  - Q/K are SBUF-resident [128, T] per head in "complex-stacked" layout:
    even head h: [qr_h(64) ; qi_h(64)], odd head h: [qi_h(64) ; qr_h(64)].
    Score contraction qr.kr + qi.ki is order-invariant, and this swap makes
    pair-packed M=128 projection matmuls land partition-aligned:
      psA = [re_h0 ; re_h1] (real weights, natural pair order)
      psB = [im_h1 ; im_h0] (imag weights, swapped pair order)
  - attn@V is ONE M=128 matmul per k-tile: v_sb head block is [vr|vi] for
    even heads, [vi|vr] for odd heads, so po rows split directly into the
    ort/oit pair layouts ([vr_even;vr_odd] / [vi_odd;vi_even]).
  - Outputs are bf16 partials; host upcasts, sums, adds output bias.
Attention math identical to baseline: S^T tiles via PE, exp on ACT (no max
subtraction: |S| <~ 4), causal mask via gpsimd affine_select, l via
ones-matmul, 1/l broadcast via K=1 matmul.
"""

import math
from contextlib import ExitStack

import numpy as np
import ml_dtypes

import concourse.bass as bass
import concourse.tile as tile
from concourse import bacc, mybir
from concourse.bass_utils import run_bass_kernel_spmd

F32 = mybir.dt.float32
BF16 = mybir.dt.bfloat16
NP_BF16 = ml_dtypes.bfloat16

# Full-problem config (hardcoded per harness contract).
CFG = dict(T=2048, D=1024, HPC=4, DH=64, TCH=512, QCH=512)
N_CORES = 8
B = 2
H_TOTAL = 16

# Flipped by test.py for profiling; harness path keeps these defaults.
TRACE = False
LAST = {}


def build_program(cfg, num_devices=N_CORES, enable_asserts=False):
    """Build the per-core SPMD Bass program. Returns nc."""
    T, D, HPC, DH = cfg["T"], cfg["D"], cfg["HPC"], cfg["DH"]
    TCH, QCH = cfg["TCH"], cfg["QCH"]
    P = 128
    DT = D // P            # din tiles
    NCH = T // TCH         # phase-1 token chunks
    TS = TCH // P          # token subtiles per chunk
    KT = T // P            # key tiles
    QC = T // QCH          # phase-2 query chunks
    QKB = QCH // P         # key tiles per query chunk step
    CW = HPC * DH          # per-core qkv width
    NPAIR = HPC // 2
    scale = 1.0 / math.sqrt(DH)

    assert DH == 64 and P == 128 and CW % 128 == 0

    nc = bacc.Bacc(
        "TRN2",
        target_bir_lowering=False,
        debug=False,
        enable_asserts=enable_asserts,
        num_devices=num_devices,
    )

    # ---- DRAM I/O (x ships pre-transposed d-major, everything bf16) ----
    xt_r = nc.dram_tensor("xt_r", [D, T], BF16, kind="ExternalInput").ap()
    xt_i = nc.dram_tensor("xt_i", [D, T], BF16, kind="ExternalInput").ap()
    wq_r = nc.dram_tensor("wq_r", [D, CW], BF16, kind="ExternalInput").ap()
    wq_i = nc.dram_tensor("wq_i", [D, CW], BF16, kind="ExternalInput").ap()
    wk_r = nc.dram_tensor("wk_r", [D, CW], BF16, kind="ExternalInput").ap()
    wk_i = nc.dram_tensor("wk_i", [D, CW], BF16, kind="ExternalInput").ap()
    wv_r = nc.dram_tensor("wv_r", [D, CW], BF16, kind="ExternalInput").ap()
    wv_i = nc.dram_tensor("wv_i", [D, CW], BF16, kind="ExternalInput").ap()
    wo_r = nc.dram_tensor("wo_r", [CW, D], BF16, kind="ExternalInput").ap()
    wo_i = nc.dram_tensor("wo_i", [CW, D], BF16, kind="ExternalInput").ap()
    bq = nc.dram_tensor("bq", [P, HPC], F32, kind="ExternalInput").ap()
    bk = nc.dram_tensor("bk", [P, HPC], F32, kind="ExternalInput").ap()
    bv_r = nc.dram_tensor("bv_r", [1, CW], BF16, kind="ExternalInput").ap()
    bv_i = nc.dram_tensor("bv_i", [1, CW], BF16, kind="ExternalInput").ap()
    out_r = nc.dram_tensor("out_r", [T, D], BF16, kind="ExternalOutput").ap()
    out_i = nc.dram_tensor("out_i", [T, D], BF16, kind="ExternalOutput").ap()

    xt_r_t = xt_r.rearrange("(n p) t -> p n t", p=P)
    xt_i_t = xt_i.rearrange("(n p) t -> p n t", p=P)
    out_r_t = out_r.rearrange("(n p) d -> p n d", p=P)
    out_i_t = out_i.rearrange("(n p) d -> p n d", p=P)

    with tile.TileContext(nc) as tc, ExitStack() as octx:
        # ---- long-lived pools ----
        const = octx.enter_context(tc.tile_pool(name="const", bufs=1))
        opool = octx.enter_context(tc.tile_pool(name="opool", bufs=1))

        ones_st = const.tile([P, P], F32)
        nc.vector.memset(ones_st, 1.0)
        ones_col = const.tile([P, 1], BF16)   # lhsT for l = ones^T @ expS
        nc.scalar.activation(ones_col, ones_st[:, 0:1],
                             mybir.ActivationFunctionType.Copy)
        ones_row = const.tile([1, P], BF16)   # lhsT for K=1 broadcasts
        nc.scalar.activation(ones_row, ones_st[0:1, :],
                             mybir.ActivationFunctionType.Copy)
        bq_sb = const.tile([P, HPC], F32)
        nc.sync.dma_start(bq_sb, bq)
        bk_sb = const.tile([P, HPC], F32)
        nc.sync.dma_start(bk_sb, bk)
        bvr_sb = const.tile([1, CW], BF16)
        nc.sync.dma_start(bvr_sb, bv_r)
        bvi_sb = const.tile([1, CW], BF16)
        nc.sync.dma_start(bvi_sb, bv_i)

        # x^T resident: [p, din_tile, T]
        xrT = opool.tile([P, DT, T], BF16, name="xrT")
        nc.sync.dma_start(xrT, xt_r_t)
        xiT = opool.tile([P, DT, T], BF16, name="xiT")
        nc.sync.dma_start(xiT, xt_i_t)

        # Q/K resident, complex-stacked per head (odd heads swapped)
        qh = [opool.tile([P, T], BF16, name=f"qh{h}") for h in range(HPC)]
        kh = [opool.tile([P, T], BF16, name=f"kh{h}") for h in range(HPC)]
        # V resident: [p, ktile, head*128 + (64|64)]; even head [vr|vi],
        # odd head [vi|vr]
        v_sb = opool.tile([P, KT, HPC * P], BF16)
        # O^T head-pair blocks, SBUF-resident into phase 3.
        # ORT[pair] rows: [vr_h_even(64) ; vr_h_odd(64)]
        # OIT[pair] rows: [vi_h_odd(64) ; vi_h_even(64)]  (host permutes wo_i)
        ort = [opool.tile([P, T], BF16, name=f"ort{p}") for p in range(NPAIR)]
        oit = [opool.tile([P, T], BF16, name=f"oit{p}") for p in range(NPAIR)]

        # ================= Phase 1: projections =================
        with ExitStack() as ctx:
            wpool = ctx.enter_context(tc.tile_pool(name="wpool", bufs=1))
            ps_qk = ctx.enter_context(tc.tile_pool(name="ps_qk", bufs=3, space="PSUM"))
            ps_v = ctx.enter_context(tc.tile_pool(name="ps_v", bufs=3, space="PSUM"))

            def load_w(ap_dram, name):
                w = wpool.tile([P, DT, CW], BF16, name=name)
                nc.sync.dma_start(
                    w, ap_dram.rearrange("(t p) m -> p t m", p=P))
                return w

            wq_r_sb = load_w(wq_r, "wq_r_sb")
            wq_i_sb = load_w(wq_i, "wq_i_sb")
            wk_r_sb = load_w(wk_r, "wk_r_sb")
            wk_i_sb = load_w(wk_i, "wk_i_sb")
            wv_r_sb = load_w(wv_r, "wv_r_sb")
            wv_i_sb = load_w(wv_i, "wv_i_sb")

            # V token-major: psum [tok(128), CW] for r and i, then pack into
            # v_sb[:, kt, head*128 + ...] with odd-head [vi|vr] swap.
            for tch in range(NCH):
                for s in range(TS):
                    ktile = tch * TS + s
                    t0 = ktile * P
                    pvr = ps_v.tile([P, CW], F32, name="pvr", tag="pv")
                    nc.tensor.matmul(pvr, ones_row, bvr_sb,
                                     start=True, stop=False)
                    for d in range(DT):
                        nc.tensor.matmul(
                            pvr, xrT[:, d, t0:t0 + P], wv_r_sb[:, d, :],
                            start=False, stop=(d == DT - 1))
                    pvi = ps_v.tile([P, CW], F32, name="pvi", tag="pv")
                    nc.tensor.matmul(pvi, ones_row, bvi_sb,
                                     start=True, stop=False)
                    for d in range(DT):
                        nc.tensor.matmul(
                            pvi, xiT[:, d, t0:t0 + P], wv_i_sb[:, d, :],
                            start=False, stop=(d == DT - 1))
                    for h in range(HPC):
                        lo = h % 2
                        dst_r = v_sb[:, ktile, h * P + 64 * lo:
                                     h * P + 64 * lo + 64]
                        dst_i = v_sb[:, ktile, h * P + 64 * (1 - lo):
                                     h * P + 64 * (1 - lo) + 64]
                        nc.any.tensor_copy(
                            out=dst_r, in_=pvr[:, h * DH:(h + 1) * DH])
                        nc.any.tensor_copy(
                            out=dst_i, in_=pvi[:, h * DH:(h + 1) * DH])

            # Q/K pair-packed: psA = [re_h0;re_h1], psB = [im_h1;im_h0]
            for tch in range(NCH):
                cs = slice(tch * TCH, (tch + 1) * TCH)
                for pr in range(NPAIR):
                    h0, h1 = 2 * pr, 2 * pr + 1
                    mA = slice(pr * P, pr * P + P)          # natural order
                    for (wr, wi, bias, dst) in (
                        (wq_r_sb, wq_i_sb, bq_sb, qh),
                        (wk_r_sb, wk_i_sb, bk_sb, kh),
                    ):
                        psA = ps_qk.tile([P, TCH], F32, name="psA", tag="psqk")
                        psB = ps_qk.tile([P, TCH], F32, name="psB", tag="psqk")
                        for d in range(DT):
                            nc.tensor.matmul(
                                psA, wr[:, d, mA], xrT[:, d, cs],
                                start=(d == 0), stop=(d == DT - 1))
                            # host swapped the imag pair columns -> [h1, h0]
                            nc.tensor.matmul(
                                psB, wi[:, d, mA], xiT[:, d, cs],
                                start=(d == 0), stop=(d == DT - 1))
                        nc.any.tensor_scalar_add(
                            out=dst[h0][0:64, cs], in0=psA[0:64],
                            scalar1=bias[0:64, h0:h0 + 1])
                        nc.any.tensor_scalar_add(
                            out=dst[h1][64:128, cs], in0=psA[64:128],
                            scalar1=bias[64:128, h1:h1 + 1])
                        nc.any.tensor_scalar_add(
                            out=dst[h1][0:64, cs], in0=psB[0:64],
                            scalar1=bias[0:64, h1:h1 + 1])
                        nc.any.tensor_scalar_add(
                            out=dst[h0][64:128, cs], in0=psB[64:128],
                            scalar1=bias[64:128, h0:h0 + 1])

        # ================= Phase 2: causal attention =================
        with ExitStack() as ctx:
            epool = ctx.enter_context(tc.tile_pool(name="epool", bufs=6))
            rpool = ctx.enter_context(tc.tile_pool(name="rpool", bufs=2))
            ps_s = ctx.enter_context(tc.tile_pool(name="ps_s", bufs=3, space="PSUM"))
            ps_o = ctx.enter_context(tc.tile_pool(name="ps_o", bufs=2, space="PSUM"))
            ps_l = ctx.enter_context(tc.tile_pool(name="ps_l", bufs=2, space="PSUM"))
            ps_b = ctx.enter_context(tc.tile_pool(name="ps_b", bufs=1, space="PSUM"))

            for h in range(HPC):
                pair, lo = h // 2, h % 2
                base_r = 64 * lo          # vr rows in ORT[pair]
                base_i = 64 * (1 - lo)    # vi rows in OIT[pair] (swapped)
                for j in range(QC):
                    nk = (j + 1) * QKB
                    qs = slice(j * QCH, (j + 1) * QCH)
                    po = ps_o.tile([P, QCH], F32, name="po")
                    pl = ps_l.tile([1, QCH], F32, name="pl")
                    for k in range(nk):
                        st = ps_s.tile([P, QCH], F32, name="st")
                        nc.tensor.matmul(
                            st, kh[h][:, k * P:(k + 1) * P], qh[h][:, qs],
                            start=True, stop=True)
                        et = epool.tile([P, QCH], BF16, name="et")
                        nc.scalar.activation(
                            et, st, mybir.ActivationFunctionType.Exp,
                            scale=scale)
                        if k >= j * QKB:
                            # keep where qtok >= ktok:
                            #   -p + f + (QCH*j - 128*k) >= 0
                            nc.gpsimd.affine_select(
                                out=et, in_=et,
                                compare_op=mybir.AluOpType.is_ge,
                                fill=0.0,
                                base=QCH * j - P * k,
                                pattern=[[1, QCH]],
                                channel_multiplier=-1)
                        nc.tensor.matmul(
                            pl, ones_col, et,
                            start=(k == 0), stop=(k == nk - 1))
                        # po rows: even head [o_r;o_i], odd head [o_i;o_r]
                        nc.tensor.matmul(
                            po, v_sb[:, k, h * P:(h + 1) * P], et,
                            start=(k == 0), stop=(k == nk - 1))
                    rl = rpool.tile([1, QCH], BF16, name="rl")
                    with nc.allow_low_precision(
                            reason="1/l in bf16 feeds bf16 bcast matmul"):
                        nc.vector.reciprocal(rl, pl)
                    pb = ps_b.tile([P, QCH], F32, name="pb")
                    nc.tensor.matmul(pb, ones_row, rl, start=True, stop=True)
                    sb_b = rpool.tile([P, QCH], BF16, name="sb_b")
                    nc.any.tensor_copy(out=sb_b, in_=pb)
                    nc.any.tensor_mul(
                        out=ort[pair][base_r:base_r + 64, qs],
                        in0=po[64 * lo:64 * lo + 64],
                        in1=sb_b[64 * lo:64 * lo + 64])
                    nc.any.tensor_mul(
                        out=oit[pair][base_i:base_i + 64, qs],
                        in0=po[64 * (1 - lo):64 * (1 - lo) + 64],
                        in1=sb_b[64 * (1 - lo):64 * (1 - lo) + 64])

        # ================= Phase 3: output projection =================
        with ExitStack() as ctx:
            wop = ctx.enter_context(tc.tile_pool(name="wop", bufs=1))
            sout = ctx.enter_context(tc.tile_pool(name="sout", bufs=3))
            ps_f = ctx.enter_context(tc.tile_pool(name="ps_f", bufs=2, space="PSUM"))

            wor_sb = wop.tile([P, NPAIR, D], BF16, name="wor_sb")
            nc.sync.dma_start(
                wor_sb, wo_r.rearrange("(t p) m -> p t m", p=P))
            woi_sb = wop.tile([P, NPAIR, D], BF16, name="woi_sb")
            nc.sync.dma_start(
                woi_sb, wo_i.rearrange("(t p) m -> p t m", p=P))

            NC2 = D // 512
            for (oblocks, wsb, odst) in (
                (ort, wor_sb, out_r_t), (oit, woi_sb, out_i_t)
            ):
                for t in range(KT):
                    for n in range(NC2):
                        pf = ps_f.tile([P, 512], F32, name="pf")
                        for kk in range(NPAIR):
                            nc.tensor.matmul(
                                pf,
                                oblocks[kk][:, t * P:(t + 1) * P],
                                wsb[:, kk, n * 512:(n + 1) * 512],
                                start=(kk == 0), stop=(kk == NPAIR - 1))
                        ot = sout.tile([P, 512], BF16, name="ot")
                        nc.any.tensor_copy(out=ot, in_=pf)
                        nc.sync.dma_start(
                            odst[:, t, n * 512:(n + 1) * 512], ot)

    nc.compile()
    return nc


def make_core_inputs(inputs, cfg=CFG):
    """Slice full inputs into 8 per-core input maps (bf16, pre-transposed x,
    pair-swapped imag weight columns, odd-head-swapped biases)."""
    HPC, DH = cfg["HPC"], cfg["DH"]
    CW = HPC * DH
    NPAIR = HPC // 2
    f32 = lambda a: np.asarray(a, dtype=np.float32)
    bf = lambda a: np.ascontiguousarray(np.asarray(a, np.float32)).astype(NP_BF16)

    def pair_swap_cols(w):
        # [D, CW]: per pair swap the two head column blocks
        out = np.empty_like(w)
        for p in range(NPAIR):
            out[:, p * 2 * DH:p * 2 * DH + DH] = \
                w[:, p * 2 * DH + DH:p * 2 * DH + 2 * DH]
            out[:, p * 2 * DH + DH:p * 2 * DH + 2 * DH] = \
                w[:, p * 2 * DH:p * 2 * DH + DH]
        return out

    x_real, x_imag = f32(inputs["x_real"]), f32(inputs["x_imag"])
    maps = []
    for c in range(N_CORES):
        b = c // 4
        g = c % 4
        cs = slice(g * CW, (g + 1) * CW)
        bqr, bqi = f32(inputs["bqr"])[cs], f32(inputs["bqi"])[cs]
        bkr, bki = f32(inputs["bkr"])[cs], f32(inputs["bki"])[cs]

        def head_bias(br, bi):
            # col h: even [br_h; bi_h], odd [bi_h; br_h]
            cols = []
            for h in range(HPC):
                r = br[h * DH:(h + 1) * DH]
                i = bi[h * DH:(h + 1) * DH]
                cols.append(np.concatenate([r, i] if h % 2 == 0 else [i, r]))
            return np.ascontiguousarray(np.stack(cols, axis=1))

        woi = f32(inputs["Woi"])[cs, :]
        # OIT pair rows are [h_odd ; h_even] -> permute wo_i rows to match
        woi_perm = np.concatenate(
            [np.concatenate([woi[2 * p * DH + DH:2 * p * DH + 2 * DH],
                             woi[2 * p * DH:2 * p * DH + DH]])
             for p in range(NPAIR)])
        maps.append({
            "xt_r": bf(x_real[b].T), "xt_i": bf(x_imag[b].T),
            "wq_r": bf(f32(inputs["Wqr"])[:, cs]),
            "wq_i": bf(pair_swap_cols(f32(inputs["Wqi"])[:, cs])),
            "wk_r": bf(f32(inputs["Wkr"])[:, cs]),
            "wk_i": bf(pair_swap_cols(f32(inputs["Wki"])[:, cs])),
            "wv_r": bf(f32(inputs["Wvr"])[:, cs]),
            "wv_i": bf(f32(inputs["Wvi"])[:, cs]),
            "wo_r": bf(f32(inputs["Wor"])[cs, :]), "wo_i": bf(woi_perm),
            "bq": head_bias(bqr, bqi), "bk": head_bias(bkr, bki),
            "bv_r": bf(f32(inputs["bvr"])[None, cs]),
            "bv_i": bf(f32(inputs["bvi"])[None, cs]),
        })
    return maps


def kernel(**inputs):
    global LAST
    nc = build_program(CFG)
    in_maps = make_core_inputs(inputs)
    res = run_bass_kernel_spmd(
        nc, in_maps, core_ids=list(range(N_CORES)), trace=TRACE)
    LAST = {"exec_time_ns": res.exec_time_ns,
            "trace": res.instructions_and_trace,
            "profile_json": res.profile_json,
            "nc": nc}
    f = lambda a: np.asarray(a, dtype=np.float32)
    bor, boi = f(inputs["bor"]), f(inputs["boi"])
    final_r = np.stack([
        sum(f(res.results[c]["out_r"]) for c in range(4 * b, 4 * b + 4)) + bor
        for b in range(B)]).astype(np.float32)
    final_i = np.stack([
        sum(f(res.results[c]["out_i"]) for c in range(4 * b, 4 * b + 4)) + boi
        for b in range(B)]).astype(np.float32)
    return final_r, final_i
